# revision 3
# baseline (speedup 1.0000x reference)
"""KMeans assignment kernel for Trainium2 (8 NeuronCores, SPMD).

Scheme E: single fp16 matmul pass on device + margin-certified host resolve.

Device per 128-row tile (stages staggered so PE never waits on Act):
  h16 = fp16(f)                [Act]
  tp = transpose(h16) via PE identity-matmul (4x fp16, PSUM)
  h1T = copy(tp)               [Act evict, PSUM->SBUF]
  mp[n, k] = sum_d h1[n,d]*g1[d,k] - csq16[k]   (8 fp16 matmuls + 2 rank-1
             contraction-2 bias matmuls into PSUM; emitted 1 tile behind
             the transposes)
  evict upper half            [Act, PSUM->SBUF; 1 tile behind matmuls]
  tpm[j] = max(mp[j], mp[j+512])  [DVE pairmax, PSUM x SBUF]
  mv8 = top8(tpm), ix8 = argmax-of-pairs  [DVE max / max_index]
Host:
  - resolves the winning pair (j* vs j*+512) with two exact f64 dots per row
  - margin = mv0-mv1 certifies the winner against all other pairs; rows with
    margin < TAU get an exact full-row rescan (float64).
  Soundness: m_ex(k) <= m_dev(k)+E <= mv1+E for k outside the winning pair,
  and m_ex(pick) >= mv0-E, so margin >= TAU >= 2E implies pick is optimal.
  Sim on the actual input: E = max|m_dev-m_ex| ~ 0.086; TAU=0.35 rescues
  ~1.7% of rows; post-rescue mismatches = 0.

Sharding: features split over N across 8 cores; centroids replicated.
"""
import sys

sys.path.insert(0, "/opt/trn_rl_repo")

import numpy as np
from contextlib import ExitStack, nullcontext

import concourse.bacc as bacc
import concourse.mybir as mybir
from concourse import tile
from concourse.bass_utils import run_bass_kernel_spmd
from concourse.masks import make_identity

N, D, K = 131072, 512, 1024
N_CORES = 8
N_PER_CORE = N // N_CORES          # 16384
N_TILES = N_PER_CORE // 128        # 128 row-tiles per core
ND = D // 128                      # 4 contraction chunks
TAU = 0.35                         # margin threshold (score units)
F32 = mybir.dt.float32
F16 = mybir.dt.float16
U32 = mybir.dt.uint32

_cached = {}
SHIP_KW = {}


def build_bass(n_tiles: int = N_TILES, repeat: int = 1):
    nc = bacc.Bacc()
    feat = nc.declare_dram_parameter("features", [n_tiles * 128, D], F32,
                                     isOutput=False)
    g1 = nc.declare_dram_parameter("g1", [D, K], F16, isOutput=False)
    b16 = nc.declare_dram_parameter("b16", [2, K], F16, isOutput=False)
    mv_o = nc.declare_dram_parameter("mv_o", [128, n_tiles * 8], F32,
                                     isOutput=True)
    ix_o = nc.declare_dram_parameter("ix_o", [128, n_tiles * 8], U32,
                                     isOutput=True)

    with tile.TileContext(nc) as tc, ExitStack() as ctx:
        const = ctx.enter_context(tc.tile_pool(name="const", bufs=1))
        work = ctx.enter_context(tc.tile_pool(name="work", bufs=4))
        red = ctx.enter_context(tc.tile_pool(name="red", bufs=3))
        ps = ctx.enter_context(tc.tile_pool(name="ps", bufs=3, space="PSUM"))
        psA = ctx.enter_context(tc.tile_pool(name="psA", bufs=2, space="PSUM"))

        g1t = const.tile([128, ND, K], F16)
        nc.sync.dma_start(out=g1t[:],
                          in_=g1[:].rearrange("(a p) k -> p a k", p=128))
        tb = const.tile([2, K], F16)
        nc.sync.dma_start(out=tb[:], in_=b16[:])
        ones2 = const.tile([2, 128], F16)
        nc.vector.memset(ones2[:], 1.0)
        ident = const.tile([128, 128], F32)
        make_identity(nc, ident[:])
        ident16 = const.tile([128, 128], F16)
        nc.vector.tensor_copy(out=ident16[:], in_=ident[:])
        mv8 = const.tile([128, n_tiles * 8], F32, tag="mv8")
        ix8 = const.tile([128, n_tiles * 8], U32, tag="ix8")

        h1Ts, mps = {}, {}

        def stage_t(rt):  # dma + fp16 cast + transposes + h1T evict
            ftile = work.tile([128, D], F32, tag="ftile")
            nc.sync.dma_start(out=ftile[:],
                              in_=feat[rt * 128:(rt + 1) * 128, :])
            th16 = work.tile([128, D], F16, tag="th16")
            nc.scalar.copy(out=th16[:], in_=ftile[:])
            tp = psA.tile([128, D], F16, tag="tp")
            for ci in range(ND):
                nc.tensor.transpose(tp[:, ci * 128:(ci + 1) * 128],
                                    th16[:, ci * 128:(ci + 1) * 128],
                                    ident16[:])
            h1T = work.tile([128, D], F16, tag="h1T")
            nc.scalar.copy(out=h1T[:], in_=tp[:])
            h1Ts[rt] = h1T

        def stage_m(rt):  # matmuls
            h1T = h1Ts.pop(rt)
            mp = ps.tile([128, K], F32, tag="mp")
            mps[rt] = mp
            for ci in range(ND):
                for kh in range(2):
                    nc.tensor.matmul(
                        mp[:, kh * 512:(kh + 1) * 512],
                        lhsT=h1T[:, ci * 128:(ci + 1) * 128],
                        rhs=g1t[:, ci, kh * 512:(kh + 1) * 512],
                        start=(ci == 0), stop=False)
            for kh in range(2):
                nc.tensor.matmul(
                    mp[:, kh * 512:(kh + 1) * 512],
                    lhsT=ones2[:],
                    rhs=tb[:, kh * 512:(kh + 1) * 512],
                    start=False, stop=True)

        def stage_b(rt):  # drain + argmax
            mp = mps.pop(rt)
            tmu = red.tile([128, 512], F32, tag="tmu")
            nc.scalar.copy(out=tmu[:], in_=mp[:, 512:1024])
            tpm = red.tile([128, 512], F32, tag="tpm")
            nc.vector.tensor_tensor(out=tpm[:], in0=mp[:, 0:512],
                                    in1=tmu[:], op=mybir.AluOpType.max)
            nc.vector.max(mv8[:, rt * 8:(rt + 1) * 8], tpm[:])
            nc.vector.max_index(ix8[:, rt * 8:(rt + 1) * 8],
                                mv8[:, rt * 8:(rt + 1) * 8], tpm[:])

        loop_ctx = tc.For_i(0, repeat, 1) if repeat > 1 else nullcontext()
        with loop_ctx:
            for rt in range(n_tiles + 2):
                if rt < n_tiles:
                    stage_t(rt)
                if 1 <= rt < n_tiles + 1:
                    stage_m(rt - 1)
                if rt >= 2:
                    stage_b(rt - 2)

        nc.sync.dma_start(out=mv_o[:], in_=mv8[:])
        nc.sync.dma_start(out=ix_o[:], in_=ix8[:])

    nc.finalize()
    return nc


def _get_nc():
    if "nc" not in _cached:
        _cached["nc"] = build_bass(**SHIP_KW)
    return _cached["nc"]


def host_inputs(centroids: np.ndarray):
    c2 = (2.0 * centroids).astype(np.float32)
    g1 = c2.astype(np.float16)                       # [D, K]
    csq = (centroids.astype(np.float64) ** 2).sum(0)
    b1 = (-csq).astype(np.float16)
    b2 = (-csq - b1.astype(np.float64)).astype(np.float16)
    b16 = np.stack([b1, b2], axis=0)                 # [2, K]
    return g1, b16, csq


def kernel(features: np.ndarray, centroids: np.ndarray) -> np.ndarray:
    features = np.ascontiguousarray(np.asarray(features, dtype=np.float32))
    centroids = np.ascontiguousarray(np.asarray(centroids, dtype=np.float32))
    g1, b16, csq = host_inputs(centroids)

    nc = _get_nc()
    in_maps = [
        {"features": features[c * N_PER_CORE:(c + 1) * N_PER_CORE],
         "g1": g1, "b16": b16}
        for c in range(N_CORES)
    ]
    res = run_bass_kernel_spmd(nc, in_maps, list(range(N_CORES))).results

    # reassemble per-row (n = core*16384 + rt*128 + p) -> [N, 8]
    mv = np.concatenate([res[c]["mv_o"].reshape(128, N_TILES, 8)
                         .transpose(1, 0, 2).reshape(N_PER_CORE, 8)
                         for c in range(N_CORES)], axis=0)
    ix = np.concatenate([res[c]["ix_o"].reshape(128, N_TILES, 8)
                         .transpose(1, 0, 2).reshape(N_PER_CORE, 8)
                         for c in range(N_CORES)], axis=0)

    jstar = ix[:, 0].astype(np.int64)
    mv0 = mv[:, 0].astype(np.float64)
    margin = mv0 - mv[:, 1].astype(np.float64)

    # exact pair resolve (two f64 dots per row, chunked)
    c2T_64 = (2.0 * centroids.astype(np.float64)).T    # [K, D]
    mA = np.empty(N); mB = np.empty(N)
    for s in range(0, N, 16384):
        e = s + 16384
        fs = features[s:e].astype(np.float64)
        ja = jstar[s:e]
        mA[s:e] = np.einsum("nd,nd->n", fs, c2T_64[ja]) - csq[ja]
        mB[s:e] = np.einsum("nd,nd->n", fs, c2T_64[ja + 512]) - csq[ja + 512]
    pick = np.where(mA >= mB, jstar, jstar + 512)
    best_pair = np.maximum(mA, mB)

    # adaptive safety: widen TAU if device error larger than sim predicted
    e_obs = np.abs(mv0 - best_pair).max()
    tau = max(TAU, 4.0 * e_obs)

    rescue = margin < tau
    if rescue.any():
        mr = (features[rescue].astype(np.float64) @ c2T_64.T) - csq[None, :]
        pick[rescue] = np.argmax(mr, axis=1)

    return pick[:, None].astype(np.float32)


def _self_test():
    rng = np.random.default_rng(0)
    f = rng.standard_normal((N, D)).astype(np.float32)
    c = rng.standard_normal((D, K)).astype(np.float32)
    out = kernel(f, c)
    x = f.astype(np.float64) @ c.astype(np.float64)
    ref = (-2 * x + (c.astype(np.float64) ** 2).sum(0)).argmin(1)
    print("mismatch:", (out[:, 0] != ref).sum(), "/", N)


if __name__ == "__main__":
    _self_test()
